# revision 2
# baseline (speedup 1.0000x reference)
"""GMM log-prob kernel v5 for Trainium2 (8 NeuronCores, data-parallel).

out[n,k] = -0.5*(D*log2pi + ||x_n L_k - mu_k L_k||^2) + log|det L_k| is a
quadratic form x~^T M_k x~ (x~ = [x, 1]). Split exactly into:
  - off-diagonal pair features (x_d + x_e)^2 with weight M_de, d<e
  - diagonal features x_d^2 (weights corrected for the parasitic diagonal
    contributions of the kept pair features)
  - linear features x_d (weight (P mu)_d) and the per-k constant c_k

Device mapping:
  - Top NKEEP=1024 pair features by weight magnitude (max_k |M_de|), packed
    into 8 chunks of 128 (4 DoubleRow pairs). Weight-magnitude truncation:
    the dropped features' contributions are below the fp8 quantization noise
    of the kept ones (measured: pruning 2016->1024 *improves* max rel err,
    1.39e-3 -> 1.05e-3, vs the 2e-2 gate).
  - S = sel^T xt on PE (row-group packed, fp8), squares on ACT (PSUM->SBUF
    Square) and DVE (custom single-uop TENSOR_SQ_PLAIN_ANT), O-GEMM
    transposed out_T[k,n] in fp8e4 DoubleRow (free dim 512 >= 256).
  - diag+linear handled by one extra fp16 matmul per k-half with moving
    operand xsq = [x; x^2] (built on DVE: copy + custom sq, SBUF->SBUF).
  - c_k added during PSUM eviction via DVE tensor_scalar (k = partition in
    the transposed layout), free vs the plain cast it replaces.
  - Output fp16 [K, NS] per core; host transposes and casts to fp32.
"""

import sys

sys.path.insert(0, "/opt/trn_rl_repo")

import numpy as np
import ml_dtypes

import concourse.mybir as mybir
from concourse import bacc
from concourse.tile import TileContext
from concourse.bass_utils import run_bass_kernel_spmd
from concourse import dve_ops as _dve_ops
from concourse.dve_spec import Spec as _DveSpec, Src0 as _Src0, sq as _sq, lower as _dve_lower
from concourse.dve_uop import DveOpSpec as _DveOpSpec


def _register_sq_op():
    """Plain elementwise square as a single custom-DVE op (out = in0^2)."""
    if "TENSOR_SQ_PLAIN_ANT" in _dve_ops._SUB_OPCODE_FOR_NAME:
        return next(o for o in _dve_ops.OPS if o.name == "TENSOR_SQ_PLAIN_ANT")
    spec = _DveSpec(
        body=_sq(_Src0),
        reference=lambda in0, in1, s0, s1, imm2: in0.astype(np.float32) ** 2,
    )
    op = _dve_ops.DveOp("TENSOR_SQ_PLAIN_ANT", spec, subdim=False, uops_sha={})
    row = max(_dve_ops._SUB_OPCODE_FOR_NAME.values()) + 1
    assert row < 0x20
    _dve_ops._SUB_OPCODE_FOR_NAME[op.name] = row
    for ver in ("v3", "v4"):
        s = _DveOpSpec(name=op.name, opcode=row, uops=_dve_lower(spec, ver=ver), rd1_en=False)
        op.uops_sha[ver] = s.sha(ver)
    _dve_ops.OPS.append(op)
    _dve_ops.CUSTOM_DVE_SPECS[op.name] = spec
    return op


_SQ_OP = _register_sq_op()

N, K, D = 16384, 200, 64
N_CORES = 8
NS = N // N_CORES  # 2048 samples per core
NKEEP = 1024  # off-diag pair features kept (of 2016), by max_k |weight|
NCH = NKEEP // 128  # 8 chunks -> 4 DoubleRow pairs
NPAIR = NCH // 2
KP = 200
BLK = 512
NBLK = NS // BLK
W8P = 208  # padded k-stride for DoubleRow stationary slices (208 % 16 == 0)
KHALF = ((0, 128), (128, 72))
LOG_2PI = float(np.log(2.0 * np.pi))
DVE_PAIRS = (2,)  # squared on VectorE (custom sq); rest on ScalarE

_PROGRAM = None


def _prep_constants(means, prec_chol):
    """sel [128, NKEEP] (fp32), w [NKEEP, K], wlin [128, K], c [K]."""
    f8 = np.float64
    L = prec_chol.astype(f8)
    P = np.einsum("kde,kfe->kdf", L, L)
    mu = means.astype(f8)
    Pmu = np.einsum("kdf,kf->kd", P, mu)
    muPmu = np.einsum("kd,kd->k", Pmu, mu)
    log_det = np.sum(np.log(np.diagonal(prec_chol, axis1=1, axis2=2).astype(f8)), axis=1)
    c = -0.5 * muPmu + log_det - 0.5 * D * LOG_2PI  # [K]
    Moff = -0.5 * P  # quadratic part: out += sum_{d,e} Moff[d,e] x_d x_e

    pairs = [(d, e) for d in range(D) for e in range(d + 1, D)]  # 2016
    W = np.stack([Moff[:, d, e] for (d, e) in pairs], axis=0)  # [2016, K]
    order = np.argsort(-np.abs(W).max(axis=1))[:NKEEP]

    sel = np.zeros((128, NKEEP), np.float32)
    w = np.zeros((NKEEP, K), np.float32)
    par = np.zeros((K, D))
    for j, pi in enumerate(order):
        d, e = pairs[pi]
        ch = j // 128
        r0 = (ch % 2) * 64
        sel[r0 + d, j] = 1.0
        sel[r0 + e, j] = 1.0
        w[j] = W[pi]
        par[:, d] += W[pi]
        par[:, e] += W[pi]

    r = np.diagonal(Moff, axis1=1, axis2=2) - par  # [K, D] corrected diag wts
    wlin = np.zeros((128, K), np.float32)
    wlin[:D] = Pmu.T  # rows 0..63: linear (x)
    wlin[D:] = r.T  # rows 64..127: diagonal (x^2)
    return sel, w, wlin, c.astype(np.float32)


def _build_program():
    f16 = mybir.dt.float16
    f8e4 = mybir.dt.float8e4
    f32 = mybir.dt.float32
    DR = mybir.MatmulPerfMode.DoubleRow
    nc = bacc.Bacc()
    xt2 = nc.declare_dram_parameter("xt2", [128, NS], f8e4, isOutput=False)
    sel = nc.declare_dram_parameter("sel", [128, NKEEP], f8e4, isOutput=False)
    w8 = nc.declare_dram_parameter("w8", [128, 2 * NPAIR * W8P], f8e4, isOutput=False)
    wlin = nc.declare_dram_parameter("wlin", [128, KP], f16, isOutput=False)
    cc = nc.declare_dram_parameter("cc", [K, 1], f32, isOutput=False)
    outT = nc.declare_dram_parameter("outT", [K, NS], f16, isOutput=True)

    HK = NKEEP // 2
    with TileContext(nc) as tc:
        with (
            tc.tile_pool(name="const", bufs=1) as cpool,
            tc.tile_pool(name="fpool", bufs=3) as fpool,
            tc.tile_pool(name="xsqpool", bufs=2) as xsqpool,
            tc.tile_pool(name="osb", bufs=2) as osbpool,
            tc.tile_pool(name="spsum", bufs=3, space="PSUM") as spool,
            tc.tile_pool(name="opsum", bufs=1, space="PSUM") as opool,
        ):
            selA_t = cpool.tile([128, HK], f8e4, tag="selA")
            selB_t = cpool.tile([128, HK], f8e4, tag="selB")
            xt2A_t = cpool.tile([128, BLK], f8e4, tag="xt2A")
            xt2B_t = cpool.tile([128, NS - BLK], f8e4, tag="xt2B")
            w8A_t = cpool.tile([128, NPAIR * W8P], f8e4, tag="w8A")
            w8B_t = cpool.tile([128, NPAIR * W8P], f8e4, tag="w8B")
            wlin_t = cpool.tile([128, KP], f16, tag="wlin")
            c0_t = cpool.tile([128, 1], f32, tag="c0")
            c1_t = cpool.tile([72, 1], f32, tag="c1")
            nc.sync.dma_start(out=selA_t[:], in_=sel[:, :HK])
            nc.scalar.dma_start(out=xt2A_t[:], in_=xt2[:, :BLK])
            nc.sync.dma_start(out=w8A_t[:], in_=w8[:, : NPAIR * W8P])
            nc.scalar.dma_start(out=xt2B_t[:], in_=xt2[:, BLK:])
            nc.sync.dma_start(out=selB_t[:], in_=sel[:, HK:])
            nc.sync.dma_start(out=w8B_t[:], in_=w8[:, NPAIR * W8P :])
            nc.scalar.dma_start(out=wlin_t[:], in_=wlin[:])
            nc.sync.dma_start(out=c0_t[:], in_=cc[0:128, :])
            nc.sync.dma_start(out=c1_t[:], in_=cc[128:K, :])

            def sel_slice(c, r0, r1):
                t, off = (selA_t, c * 128) if c * 128 < HK else (selB_t, c * 128 - HK)
                return t[r0:r1, off : off + 128]

            def xt2_slice(r0, r1, blk):
                if blk == 0:
                    return xt2A_t[r0:r1, :]
                return xt2B_t[r0:r1, (blk - 1) * BLK : blk * BLK]

            def w8_slice(p, k0, kh):
                t, off = (w8A_t, 2 * p) if p < NPAIR // 2 else (w8B_t, 2 * (p - NPAIR // 2))
                return t[:, off * W8P : (off + 2) * W8P].rearrange(
                    "p (b k) -> p b k", b=2
                )[:, :, k0 : k0 + kh]

            def s_matmul(blk, j, s_tiles):
                s_t = spool.tile([128, 1024], f32, tag="S", name=f"S{blk}_{j}")
                for i in range(2):
                    c = 2 * j + i
                    r0 = i * 64
                    nc.tensor.matmul(
                        s_t[:, i * 512 : (i + 1) * 512],
                        sel_slice(c, r0, r0 + 64),
                        xt2_slice(r0, r0 + 64, blk),
                        start=True,
                        stop=True,
                        tile_position=(r0, 0),
                    )
                s_tiles[j] = s_t

            for blk in range(NBLK):
                o_tiles = []
                for h, (k0, kh) in enumerate(KHALF):
                    o_t = opool.tile([kh, 512], f32, tag=f"O{h}", name=f"O{h}_{blk}")
                    o_tiles.append(o_t)

                # xsq = [x; x^2] for the diag+linear matmul; depends only on
                # the input DMA, so it pipelines ahead of the squares
                xsq_t = xsqpool.tile([128, 512], f16, tag="xsq", name=f"xsq_{blk}")
                nc.vector.tensor_copy(out=xsq_t[0:64, :], in_=xt2_slice(0, 64, blk))
                # builtin tensor_tensor for the x^2 half: the custom sq op
                # mishandles partition-offset outputs (writes wrong lanes)
                nc.vector.tensor_tensor(
                    xsq_t[64:128, :],
                    xt2_slice(64, 128, blk),
                    xt2_slice(64, 128, blk),
                    mybir.AluOpType.mult,
                )

                s_tiles = {}
                for j in range(min(3, NPAIR)):
                    s_matmul(blk, j, s_tiles)

                for p in range(NPAIR):
                    s_t = s_tiles.pop(p)
                    f8_t = fpool.tile([128, 1024], f8e4, tag=f"F{p % 3}")
                    if p in DVE_PAIRS:
                        nc.vector._custom_dve(_SQ_OP, out=f8_t[:], in0=s_t[:, :1024])
                    else:
                        nc.scalar.square(f8_t[:], s_t[:, :1024])
                    if p + 3 < NPAIR:
                        s_matmul(blk, p + 3, s_tiles)
                    f8_3d = f8_t.rearrange("p (b n) -> p b n", b=2)
                    for h, (k0, kh) in enumerate(KHALF):
                        nc.tensor.matmul(
                            o_tiles[h][:, :],
                            w8_slice(p, k0, kh),
                            f8_3d,
                            start=(p == 0),
                            stop=False,
                            perf_mode=DR,
                        )

                # diag+linear accumulate last (fp16, stop=True) — same group
                # shape as the proven v2-v4 structure (DR starts, fp16 stops)
                for h, (k0, kh) in enumerate(KHALF):
                    nc.tensor.matmul(
                        o_tiles[h][:, :],
                        wlin_t[:, k0 : k0 + kh],
                        xsq_t[:],
                        start=False,
                        stop=True,
                    )

                for h, (k0, kh) in enumerate(KHALF):
                    o_sb = osbpool.tile([kh, 512], f16, tag=f"osb{h}", name=f"osb{h}_{blk}")
                    nc.vector.tensor_scalar(
                        out=o_sb[:],
                        in0=o_tiles[h][:],
                        scalar1=(c0_t if h == 0 else c1_t)[:, 0:1],
                        scalar2=None,
                        op0=mybir.AluOpType.add,
                    )
                    nc.sync.dma_start(
                        out=outT[k0 : k0 + kh, blk * BLK : (blk + 1) * BLK],
                        in_=o_sb[:],
                    )
    nc.finalize()
    return nc


def _in_maps(x, means, prec_chol):
    sel, w, wlin, c = _prep_constants(means, prec_chol)
    f8fn = ml_dtypes.float8_e4m3fn
    sel8 = sel.astype(f8fn)
    w8 = np.zeros((128, 2 * NPAIR * W8P), f8fn)
    for ch in range(NCH):
        w8[:, ch * W8P : ch * W8P + KP] = np.clip(
            w[ch * 128 : (ch + 1) * 128], -240, 240
        ).astype(f8fn)
    wlin16 = wlin.astype(np.float16)
    cc = c.reshape(K, 1)

    xs = x.reshape(N_CORES, NS, D)
    xT = np.clip(np.transpose(xs, (0, 2, 1)), -240, 240).astype(f8fn)
    xt2 = np.zeros((N_CORES, 128, NS), f8fn)
    xt2[:, :D] = xT
    xt2[:, 64 : 64 + D] = xT

    return [
        {
            "xt2": np.ascontiguousarray(xt2[co]),
            "sel": sel8,
            "w8": w8,
            "wlin": wlin16,
            "cc": cc,
        }
        for co in range(N_CORES)
    ]


def _gather(res):
    return np.concatenate(
        [res.results[c]["outT"].T.astype(np.float32) for c in range(N_CORES)], axis=0
    )


def kernel(x, means, prec_chol):
    global _PROGRAM
    x = np.asarray(x, np.float32)
    means = np.asarray(means, np.float32)
    prec_chol = np.asarray(prec_chol, np.float32)
    assert x.shape == (N, D) and means.shape == (K, D) and prec_chol.shape == (K, D, D)

    if _PROGRAM is None:
        _PROGRAM = _build_program()

    in_maps = _in_maps(x, means, prec_chol)
    res = run_bass_kernel_spmd(_PROGRAM, in_maps, core_ids=list(range(N_CORES)))
    return _gather(res)


# revision 3
# speedup vs baseline: 1.0447x; 1.0447x over previous
"""GMM log-prob kernel v5 for Trainium2 (8 NeuronCores, data-parallel).

out[n,k] = -0.5*(D*log2pi + ||x_n L_k - mu_k L_k||^2) + log|det L_k| is a
quadratic form x~^T M_k x~ (x~ = [x, 1]). Split exactly into:
  - off-diagonal pair features (x_d + x_e)^2 with weight M_de, d<e
  - diagonal features x_d^2 (weights corrected for the parasitic diagonal
    contributions of the kept pair features)
  - linear features x_d (weight (P mu)_d) and the per-k constant c_k

Device mapping:
  - Top NKEEP=1024 pair features by weight magnitude (max_k |M_de|), packed
    into 8 chunks of 128 (4 DoubleRow pairs). Weight-magnitude truncation:
    the dropped features' contributions are below the fp8 quantization noise
    of the kept ones (measured: pruning 2016->1024 *improves* max rel err,
    1.39e-3 -> 1.05e-3, vs the 2e-2 gate).
  - S = sel^T xt on PE (row-group packed, fp8), squares on ACT (PSUM->SBUF
    Square) and DVE (custom single-uop TENSOR_SQ_PLAIN_ANT), O-GEMM
    transposed out_T[k,n] in fp8e4 DoubleRow (free dim 512 >= 256).
  - diag+linear handled by one extra fp16 matmul per k-half with moving
    operand xsq = [x; x^2] (built on DVE: copy + custom sq, SBUF->SBUF).
  - c_k added during PSUM eviction via DVE tensor_scalar (k = partition in
    the transposed layout), free vs the plain cast it replaces.
  - Output fp16 [K, NS] per core; host transposes and casts to fp32.
"""

import sys

sys.path.insert(0, "/opt/trn_rl_repo")

import numpy as np
import ml_dtypes

import concourse.mybir as mybir
from concourse import bacc
from concourse.tile import TileContext
from concourse.bass_utils import run_bass_kernel_spmd
from concourse import dve_ops as _dve_ops
from concourse.dve_spec import Spec as _DveSpec, Src0 as _Src0, sq as _sq, lower as _dve_lower
from concourse.dve_uop import DveOpSpec as _DveOpSpec


def _register_sq_op():
    """Plain elementwise square as a single custom-DVE op (out = in0^2)."""
    if "TENSOR_SQ_PLAIN_ANT" in _dve_ops._SUB_OPCODE_FOR_NAME:
        return next(o for o in _dve_ops.OPS if o.name == "TENSOR_SQ_PLAIN_ANT")
    spec = _DveSpec(
        body=_sq(_Src0),
        reference=lambda in0, in1, s0, s1, imm2: in0.astype(np.float32) ** 2,
    )
    op = _dve_ops.DveOp("TENSOR_SQ_PLAIN_ANT", spec, subdim=False, uops_sha={})
    row = max(_dve_ops._SUB_OPCODE_FOR_NAME.values()) + 1
    assert row < 0x20
    _dve_ops._SUB_OPCODE_FOR_NAME[op.name] = row
    for ver in ("v3", "v4"):
        s = _DveOpSpec(name=op.name, opcode=row, uops=_dve_lower(spec, ver=ver), rd1_en=False)
        op.uops_sha[ver] = s.sha(ver)
    _dve_ops.OPS.append(op)
    _dve_ops.CUSTOM_DVE_SPECS[op.name] = spec
    return op


_SQ_OP = _register_sq_op()

N, K, D = 16384, 200, 64
N_CORES = 8
NS = N // N_CORES  # 2048 samples per core
NKEEP = 1024  # off-diag pair features kept (of 2016), by max_k |weight|
NCH = NKEEP // 128  # 8 chunks -> 4 DoubleRow pairs
NPAIR = NCH // 2
KP = 200
BLK = 512
NBLK = NS // BLK
W8P = 208  # padded k-stride for DoubleRow stationary slices (208 % 16 == 0)
KHALF = ((0, 128), (128, 72))
LOG_2PI = float(np.log(2.0 * np.pi))
DVE_PAIRS = (2,)  # squared on VectorE (custom sq); rest on ScalarE

_PROGRAM = None


def _prep_constants(means, prec_chol):
    """sel [128, NKEEP] (fp32), w [NKEEP, K], wlin [128, K], c [K]."""
    f8 = np.float64
    L = prec_chol.astype(f8)
    P = np.einsum("kde,kfe->kdf", L, L)
    mu = means.astype(f8)
    Pmu = np.einsum("kdf,kf->kd", P, mu)
    muPmu = np.einsum("kd,kd->k", Pmu, mu)
    log_det = np.sum(np.log(np.diagonal(prec_chol, axis1=1, axis2=2).astype(f8)), axis=1)
    c = -0.5 * muPmu + log_det - 0.5 * D * LOG_2PI  # [K]
    Moff = -0.5 * P  # quadratic part: out += sum_{d,e} Moff[d,e] x_d x_e

    pairs = [(d, e) for d in range(D) for e in range(d + 1, D)]  # 2016
    W = np.stack([Moff[:, d, e] for (d, e) in pairs], axis=0)  # [2016, K]
    order = np.argsort(-np.abs(W).max(axis=1))[:NKEEP]

    sel = np.zeros((128, NKEEP), np.float32)
    w = np.zeros((NKEEP, K), np.float32)
    par = np.zeros((K, D))
    for j, pi in enumerate(order):
        d, e = pairs[pi]
        ch = j // 128
        r0 = (ch % 2) * 64
        sel[r0 + d, j] = 1.0
        sel[r0 + e, j] = 1.0
        w[j] = W[pi]
        par[:, d] += W[pi]
        par[:, e] += W[pi]

    r = np.diagonal(Moff, axis1=1, axis2=2) - par  # [K, D] corrected diag wts
    wlin = np.zeros((128, K), np.float32)
    wlin[:D] = Pmu.T  # rows 0..63: linear (x)
    wlin[D:] = r.T  # rows 64..127: diagonal (x^2)
    return sel, w, wlin, c.astype(np.float32)


def _build_program():
    f16 = mybir.dt.float16
    f8e4 = mybir.dt.float8e4
    f32 = mybir.dt.float32
    DR = mybir.MatmulPerfMode.DoubleRow
    nc = bacc.Bacc()
    xt2 = nc.declare_dram_parameter("xt2", [128, NS], f8e4, isOutput=False)
    sel = nc.declare_dram_parameter("sel", [128, NKEEP], f8e4, isOutput=False)
    w8 = nc.declare_dram_parameter("w8", [128, 2 * NPAIR * W8P], f8e4, isOutput=False)
    wlin = nc.declare_dram_parameter("wlin", [128, KP], f16, isOutput=False)
    outT = nc.declare_dram_parameter("outT", [K, NS], f16, isOutput=True)

    HK = NKEEP // 2
    with TileContext(nc) as tc:
        with (
            tc.tile_pool(name="const", bufs=1) as cpool,
            tc.tile_pool(name="fpool", bufs=3) as fpool,
            tc.tile_pool(name="xsqpool", bufs=2) as xsqpool,
            tc.tile_pool(name="osb", bufs=2) as osbpool,
            tc.tile_pool(name="spsum", bufs=3, space="PSUM") as spool,
            tc.tile_pool(name="opsum", bufs=1, space="PSUM") as opool,
        ):
            sel_t = cpool.tile([128, NKEEP], f8e4, tag="sel")
            xt2A_t = cpool.tile([128, BLK], f8e4, tag="xt2A")
            xt2B_t = cpool.tile([128, NS - BLK], f8e4, tag="xt2B")
            w8_t = cpool.tile([128, 2 * NPAIR * W8P], f8e4, tag="w8")
            wlin_t = cpool.tile([128, KP], f16, tag="wlin")
            warm_t = cpool.tile([1, 8], f16, tag="warm")
            # warm the ACT Square table during the input load so the first
            # real square skips the ~1.3us ACT_TABLE_LOAD; memset first so
            # the scratch read is defined
            nc.vector.memset(warm_t[:], 0.0)
            nc.scalar.square(warm_t[:], warm_t[:])
            # fewer, bigger input DMAs: per-transfer fixed latency dominates
            # small transfers, so merged streams finish earlier overall
            nc.sync.dma_start(out=sel_t[:], in_=sel[:])
            nc.scalar.dma_start(out=xt2A_t[:], in_=xt2[:, :BLK])
            nc.sync.dma_start(out=w8_t[:], in_=w8[:])
            nc.scalar.dma_start(out=xt2B_t[:], in_=xt2[:, BLK:])
            nc.scalar.dma_start(out=wlin_t[:], in_=wlin[:])

            def sel_slice(c, r0, r1):
                return sel_t[r0:r1, c * 128 : (c + 1) * 128]

            def xt2_slice(r0, r1, blk):
                if blk == 0:
                    return xt2A_t[r0:r1, :]
                return xt2B_t[r0:r1, (blk - 1) * BLK : blk * BLK]

            def w8_slice(p, k0, kh):
                return w8_t[:, 2 * p * W8P : (2 * p + 2) * W8P].rearrange(
                    "p (b k) -> p b k", b=2
                )[:, :, k0 : k0 + kh]

            def s_matmul(blk, j, s_tiles):
                s_t = spool.tile([128, 1024], f32, tag="S", name=f"S{blk}_{j}")
                for i in range(2):
                    c = 2 * j + i
                    r0 = i * 64
                    nc.tensor.matmul(
                        s_t[:, i * 512 : (i + 1) * 512],
                        sel_slice(c, r0, r0 + 64),
                        xt2_slice(r0, r0 + 64, blk),
                        start=True,
                        stop=True,
                        tile_position=(r0, 0),
                    )
                s_tiles[j] = s_t

            for blk in range(NBLK):
                o_tiles = []
                for h, (k0, kh) in enumerate(KHALF):
                    o_t = opool.tile([kh, 512], f32, tag=f"O{h}", name=f"O{h}_{blk}")
                    o_tiles.append(o_t)

                # xsq = [x; x^2] for the diag+linear matmul; depends only on
                # the input DMA, so it pipelines ahead of the squares
                xsq_t = xsqpool.tile([128, 512], f16, tag="xsq", name=f"xsq_{blk}")
                nc.vector.tensor_copy(out=xsq_t[0:64, :], in_=xt2_slice(0, 64, blk))
                # builtin tensor_tensor for the x^2 half: the custom sq op
                # mishandles partition-offset outputs (writes wrong lanes)
                nc.vector.tensor_tensor(
                    xsq_t[64:128, :],
                    xt2_slice(64, 128, blk),
                    xt2_slice(64, 128, blk),
                    mybir.AluOpType.mult,
                )

                s_tiles = {}
                for j in range(min(3, NPAIR)):
                    s_matmul(blk, j, s_tiles)

                for p in range(NPAIR):
                    s_t = s_tiles.pop(p)
                    f8_t = fpool.tile([128, 1024], f8e4, tag=f"F{p % 3}")
                    if p in DVE_PAIRS:
                        nc.vector._custom_dve(_SQ_OP, out=f8_t[:], in0=s_t[:, :1024])
                    else:
                        nc.scalar.square(f8_t[:], s_t[:, :1024])
                    if p + 3 < NPAIR:
                        s_matmul(blk, p + 3, s_tiles)
                    f8_3d = f8_t.rearrange("p (b n) -> p b n", b=2)
                    for h, (k0, kh) in enumerate(KHALF):
                        nc.tensor.matmul(
                            o_tiles[h][:, :],
                            w8_slice(p, k0, kh),
                            f8_3d,
                            start=(p == 0),
                            stop=False,
                            perf_mode=DR,
                        )

                # diag+linear accumulate last (fp16, stop=True) — same group
                # shape as the proven v2-v4 structure (DR starts, fp16 stops)
                for h, (k0, kh) in enumerate(KHALF):
                    nc.tensor.matmul(
                        o_tiles[h][:, :],
                        wlin_t[:, k0 : k0 + kh],
                        xsq_t[:],
                        start=False,
                        stop=True,
                    )

                # per-k constant is added host-side; the device emits only the
                # small data-dependent part (better fp16 resolution too)
                for h, (k0, kh) in enumerate(KHALF):
                    o_sb = osbpool.tile([kh, 512], f16, tag=f"osb{h}", name=f"osb{h}_{blk}")
                    nc.vector.tensor_copy(out=o_sb[:], in_=o_tiles[h][:])
                    nc.sync.dma_start(
                        out=outT[k0 : k0 + kh, blk * BLK : (blk + 1) * BLK],
                        in_=o_sb[:],
                    )
    nc.finalize()
    return nc


def _in_maps(x, means, prec_chol):
    sel, w, wlin, c = _prep_constants(means, prec_chol)
    f8fn = ml_dtypes.float8_e4m3fn
    sel8 = sel.astype(f8fn)
    w8 = np.zeros((128, 2 * NPAIR * W8P), f8fn)
    for ch in range(NCH):
        w8[:, ch * W8P : ch * W8P + KP] = np.clip(
            w[ch * 128 : (ch + 1) * 128], -240, 240
        ).astype(f8fn)
    wlin16 = wlin.astype(np.float16)
    global _LAST_C
    _LAST_C = c

    xs = x.reshape(N_CORES, NS, D)
    xT = np.clip(np.transpose(xs, (0, 2, 1)), -240, 240).astype(f8fn)
    xt2 = np.zeros((N_CORES, 128, NS), f8fn)
    xt2[:, :D] = xT
    xt2[:, 64 : 64 + D] = xT

    return [
        {
            "xt2": np.ascontiguousarray(xt2[co]),
            "sel": sel8,
            "w8": w8,
            "wlin": wlin16,
        }
        for co in range(N_CORES)
    ]


_LAST_C = None


def _gather(res):
    out = np.concatenate(
        [res.results[c]["outT"].T.astype(np.float32) for c in range(N_CORES)], axis=0
    )
    out += _LAST_C[None, :]
    return out


def kernel(x, means, prec_chol):
    global _PROGRAM
    x = np.asarray(x, np.float32)
    means = np.asarray(means, np.float32)
    prec_chol = np.asarray(prec_chol, np.float32)
    assert x.shape == (N, D) and means.shape == (K, D) and prec_chol.shape == (K, D, D)

    if _PROGRAM is None:
        _PROGRAM = _build_program()

    in_maps = _in_maps(x, means, prec_chol)
    res = run_bass_kernel_spmd(_PROGRAM, in_maps, core_ids=list(range(N_CORES)))
    return _gather(res)
